# revision 28
# baseline (speedup 1.0000x reference)
"""Trainium2 Bass kernel for nn_DotMatrix.

Math: for each (b, ell, t) the reference computes a complex pairwise dot
matrix O[i,j] = sum_m z[i,m] * w[j,m] where z = rep[b,:,t,:,:] as complex
and w the sign-flipped conjugation partner.  As a real matmul:

  lhsT[k, i]   k = (c,m) stacked: [Zr.T; Zi.T]                 [2m, 256]
  rhs[k, 2j+c'] c'=0: [FZr; -FZi], c'=1: [FZi; FZr]            [2m, 512]
  out = lhsT.T @ rhs  -> [256 i, 512 (j,c)]

with FZr[m',j] = s[m'] * Zr[j, M-1-m'], s[m'] = (-1)^(ell+m').

Precision: bf16 operands with fp32 PSUM accumulation and fp16 stores give
~3e-3 relative error on the final output (gate is 2e-2), so no multi-term
decomposition is needed — the contraction dim stays at K = 2m <= 14, the
input tensors total just 393KB per core, and the PE streams one column
per cycle (fp16 operands would halve that rate; fp16 is only used on the
store side where it halves HBM traffic at no cost).

Symmetry trick: the pairwise matrix is symmetric in (i,j) for both the
real and imaginary components (O[i,j] = O[j,i]), so each channel only
computes 32-row i-blocks against j >= 32*bi — 56.25% of the matrix —
and the host mirrors the lower block-triangle for free.

Sharding: 8 cores = 2 batches x 4 tau-quarters.  Each core owns 32
channels ch = ell*8 + s (t = tq*8 + s).  Four channels (a quad) share
each matmul's 128 PSUM partitions via column tiling (tile_position) —
the four streams run concurrently on disjoint 32-column PE tiles, so a
quad i-block costs one W-column pass.  Each ell lives in its own
32-partition row group (rows 32*ell + [0, 2m)).  The eight i-blocks of a
quad are packed into five single-PSUM-bank tiles — (0), (1,7), (2,6),
(3,5), (4) — so evacuation is five wide copies (f32 -> fp16) with a
fixed ScalarE/VectorE split sized to each engine's measured per-column
rate.  Two quads share each [128, 4608] staging tile so output stores
are 9216B-per-partition-line DMAs (small lines throttle the SDMA
engines), issued on the sync ring with the last pair on the scalar ring.
Host reassembles the full [2,256,256,128,2] output.
"""

import numpy as np
import ml_dtypes

import concourse.bass as bass
import concourse.bacc as bacc
import concourse.mybir as mybir
from concourse.bass_utils import run_bass_kernel_spmd
from concourse.tile import TileContext

B, N, TAU, NELL = 2, 256, 32, 4
NCORES = 8
NCH = 32          # channels per core (4 ell * 8 slots)
F32 = mybir.dt.float32
F16 = mybir.dt.float16
BF16 = mybir.dt.bfloat16
BFNP = ml_dtypes.bfloat16
KS = [2 * (2 * ell + 1) for ell in range(NELL)]       # 2, 6, 10, 14
# contraction dims padded with zero rows to a multiple of 4: the PE
# streams bf16 moving data at half rate when K is not 4-aligned
KP = [4, 8, 12, 16]
BIW = [512 - 64 * bi for bi in range(8)]              # cols per 32-row i-block
# PSUM pack: two 2-bank tiles + one single-bank tile per quad.  Each
# i-block sits inside one 2KB bank; each PACK group fills its tile
# exactly (512+448+64 = 384+128+320+192 = 1024, 256) so evacuation is
# one wide copy per tile.
PACK = [(0, 1, 7), (2, 6, 3, 5), (4,)]                # widths 1024, 1024, 256
PACKW = [sum(BIW[b] for b in g) for g in PACK]
# offset of each i-block inside the quad's 2304-col staging span
BIO2 = {}
_off = 0
for _g in PACK:
    for _b in _g:
        BIO2[_b] = _off
        _off += BIW[_b]
OTW = 2304                                            # sum of all widths
QUAD_ORDER = [(0, 0), (0, 1), (3, 0), (3, 1), (2, 0), (2, 1), (1, 0), (1, 1)]
IN_COLS = 8 * 256 + 8 * 512                           # 6144: lhs slots then rhs slots

_NC_CACHE = {}


def _build_bass():
    nc = bacc.Bacc()
    # One input tensor per ell: [2m, 6144] bf16; cols [0:2048) hold the
    # eight 256-wide lhsT slot blocks, [2048:6144) the eight 512-wide rhs
    # slot blocks.  Each lands in SBUF row group 32*ell.
    inps = [
        nc.declare_dram_parameter(f"inp{e}", [KP[e], IN_COLS], BF16, isOutput=False)
        for e in range(NELL)
    ]
    # Output: eight 2304-col quad spans side by side; stores cover
    # single quads or pairs depending on position in the schedule
    out = nc.declare_dram_parameter("out", [128, 8 * OTW], F16, isOutput=True)

    with TileContext(nc) as tc:
        with (
            tc.tile_pool(name="lin", bufs=1) as lin_pool,
            tc.tile_pool(name="in0", bufs=1) as in0_pool,
            tc.tile_pool(name="in1", bufs=1) as in1_pool,
            tc.tile_pool(name="in2", bufs=1) as in2_pool,
            tc.tile_pool(name="in3", bufs=1) as in3_pool,
            tc.tile_pool(name="psA", bufs=3, space="PSUM") as psA_pool,
            tc.tile_pool(name="psB", bufs=2, space="PSUM") as psB_pool,
            tc.tile_pool(name="ot1", bufs=2) as ot1_pool,
            tc.tile_pool(name="ot2", bufs=3) as ot2_pool,
        ):
            # One SBUF tile per ell, each in its own pool: dependency
            # tracking is pool-slot-granular, so sharing a pool would make
            # the FIRST matmul wait for the LAST input DMA.  Separate
            # pools let quad 0 start as soon as ell0's load lands, with
            # later ells streaming in under compute.
            in_pools = [in0_pool, in1_pool, in2_pool, in3_pool]
            in_sbs = [
                in_pools[e].tile([128, IN_COLS], BF16, name=f"in_sb{e}")
                for e in range(NELL)
            ]
            # PE pre-warm: dependency-free dummy matmuls keep the PE busy
            # from kernel start until the first input lands — a PE idle
            # gap here drops the HAM clock gate and the whole kernel then
            # streams at half rate.
            warm_in = lin_pool.tile([128, 512], BF16, name="warm_in")
            warm_ps = psA_pool.tile([128, 1024], F32, tag="psA", name="warm_ps")
            nc.vector.memset(warm_in[:], 0.0)
            # The first real matmul cannot start before ~11.8us (input
            # completion semaphore latency is fixed regardless of input
            # size); eight warmups bridge the whole window so the PE never
            # idles and the clock stays at full rate.
            for _ in range(8):
                nc.tensor.matmul(
                    warm_ps[:, 0:512], warm_in[:, 0:128], warm_in[:, 0:512],
                    start=True, stop=True,
                )
            # Split the four input loads across both HWDGE rings so their
            # ~700ns descriptor generations run in parallel: every matmul
            # waits on the full input set, so last-input-landed is what
            # gates compute start.
            nc.sync.dma_start(out=in_sbs[0][0 : KP[0], :], in_=inps[0][:])
            nc.scalar.dma_start(out=in_sbs[3][0 : KP[3], :], in_=inps[3][:])
            nc.sync.dma_start(out=in_sbs[2][0 : KP[2], :], in_=inps[2][:])
            nc.scalar.dma_start(out=in_sbs[1][0 : KP[1], :], in_=inps[1][:])
            # store schedule: (0), (1,2), (3,4), (5,6), (7) — the early
            # single gets the store stream flowing ~2us sooner, the pairs
            # keep 9216B partition lines for full SDMA packet efficiency
            ot = None
            for qidx, (e, v) in enumerate(QUAD_ORDER):
                K = KP[e]
                in_sb = in_sbs[e]
                if qidx in (0, 7):
                    ot = ot1_pool.tile([128, OTW], F16)
                    qoff = 0
                elif qidx in (1, 3, 5):
                    ot = ot2_pool.tile([128, 2 * OTW], F16)
                    qoff = 0
                else:
                    qoff = OTW
                for gi, grp in enumerate(PACK):
                    if gi == 2:
                        ps = psB_pool.tile([128, 512], F32, name="psb")
                    else:
                        ps = psA_pool.tile([128, 1024], F32, tag="psA", name="psa")
                    poff = 0
                    for bi in grp:
                        W = BIW[bi]
                        for c4 in range(4):  # channel within quad
                            sl = v * 4 + c4
                            lo = sl * 256
                            ro = 2048 + sl * 512
                            nc.tensor.matmul(
                                ps[c4 * 32 : (c4 + 1) * 32, poff : poff + W],
                                in_sb[0:K, lo + bi * 32 : lo + bi * 32 + 32],
                                in_sb[0:K, ro + 64 * bi : ro + 512],
                                start=True,
                                stop=True,
                                tile_position=(0, c4 * 32),
                            )
                        poff += W
                    base = qoff + BIO2[grp[0]]
                    # one wide copy per tile; scalar (the faster PSUM
                    # reader) takes tile 0 + the small tile, vector tile 1
                    if gi == 1:
                        nc.vector.tensor_copy(
                            out=ot[:, base : base + PACKW[gi]], in_=ps[:, 0 : PACKW[gi]]
                        )
                    else:
                        nc.scalar.copy(
                            ot[:, base : base + PACKW[gi]], ps[:, 0 : PACKW[gi]]
                        )
                if qidx in (0, 2, 4, 6, 7):
                    # sync ring carries everything except the final single,
                    # which rides the scalar ring (its copies are done by
                    # then) so the last two transfers drain concurrently
                    span = OTW if qidx in (0, 7) else 2 * OTW
                    dst0 = qidx * OTW if qidx in (0, 7) else (qidx - 1) * OTW
                    eng = nc.scalar if qidx == 7 else nc.sync
                    eng.dma_start(out=out[:, dst0 : dst0 + span], in_=ot[:])
    nc.compile()
    return nc


def _host_prep(reps, cid):
    """Build per-core bf16 lhsT/rhs input tensors (one per ell)."""
    b, tq = cid // 4, cid % 4
    im = {}
    for ell in range(NELL):
        rep = reps[ell]
        m = 2 * ell + 1
        s_vec = ((-1.0) ** (ell + np.arange(m))).astype(np.float32)
        arr = np.zeros((KP[ell], IN_COLS), np.float32)
        for sidx in range(8):
            t = tq * 8 + sidx
            Z = rep[b, :, t]                      # [256, m, 2]
            Zr, Zi = Z[..., 0], Z[..., 1]         # [256, m]
            arr[0:m, sidx * 256 : sidx * 256 + 256] = Zr.T
            arr[m : 2 * m, sidx * 256 : sidx * 256 + 256] = Zi.T
            FZr = s_vec[:, None] * Zr[:, ::-1].T             # [m, 256]
            FZi = s_vec[:, None] * Zi[:, ::-1].T
            R = np.empty((2 * m, 256, 2), np.float32)
            R[0:m, :, 0] = FZr
            R[m:, :, 0] = -FZi
            R[0:m, :, 1] = FZi
            R[m:, :, 1] = FZr
            ro = 2048 + sidx * 512
            arr[0 : 2 * m, ro : ro + 512] = R.reshape(2 * m, 512)
        im[f"inp{ell}"] = arr.astype(BFNP)
    return im


def _run(in_maps, **kw):
    if "nc" not in _NC_CACHE:
        _NC_CACHE["nc"] = _build_bass()
    return run_bass_kernel_spmd(_NC_CACHE["nc"], in_maps, list(range(NCORES)), **kw)


def kernel(rep0, rep1, rep2, rep3, _bass_kw=None):
    reps = [np.ascontiguousarray(np.asarray(r, dtype=np.float32)) for r in (rep0, rep1, rep2, rep3)]
    in_maps = [_host_prep(reps, cid) for cid in range(NCORES)]
    res = _run(in_maps, **(_bass_kw or {}))
    out = np.empty((B, N, N, NELL * TAU, 2), np.float32)
    for cid in range(NCORES):
        b, tq = cid // 4, cid % 4
        arr = res.results[cid]["out"]          # [128, 8*2304] fp16
        o = np.empty((NELL, 8, 256, 256, 2), np.float32)   # [ell, slot, i, j, c]
        for qidx, (e, v) in enumerate(QUAD_ORDER):
            a = arr[:, qidx * OTW : (qidx + 1) * OTW].astype(np.float32)
            for bi in range(8):
                nj = 256 - 32 * bi
                blk = a[:, BIO2[bi] : BIO2[bi] + BIW[bi]].reshape(4, 32, nj, 2)
                for c4 in range(4):
                    o[e, 4 * v + c4, 32 * bi : 32 * bi + 32, 32 * bi :, :] = blk[c4]
        for bi in range(1, 8):                  # mirror lower block triangle
            r = slice(32 * bi, 32 * bi + 32)
            o[:, :, r, : 32 * bi, :] = o[:, :, : 32 * bi, r, :].transpose(0, 1, 3, 2, 4)
        for e in range(NELL):
            lo = e * TAU + tq * 8
            out[b, :, :, lo : lo + 8, :] = o[e].transpose(1, 2, 0, 3)
    kernel.last_result = res
    return out


# revision 33
# speedup vs baseline: 1.0303x; 1.0303x over previous
"""Trainium2 Bass kernel for nn_DotMatrix.

Math: for each (b, ell, t) the reference computes a complex pairwise dot
matrix O[i,j] = sum_m z[i,m] * w[j,m] where z = rep[b,:,t,:,:] as complex
and w the sign-flipped conjugation partner.  As a real matmul:

  lhsT[k, i]   k = (c,m) stacked: [Zr.T; Zi.T]                 [2m, 256]
  rhs[k, 2j+c'] c'=0: [FZr; -FZi], c'=1: [FZi; FZr]            [2m, 512]
  out = lhsT.T @ rhs  -> [256 i, 512 (j,c)]

with FZr[m',j] = s[m'] * Zr[j, M-1-m'], s[m'] = (-1)^(ell+m').

Precision: bf16 operands with fp32 PSUM accumulation and fp16 stores give
~3e-3 relative error on the final output (gate is 2e-2), so no multi-term
decomposition is needed — the contraction dim stays at K = 2m <= 14, the
input tensors total just 393KB per core, and the PE streams one column
per cycle (fp16 operands would halve that rate; fp16 is only used on the
store side where it halves HBM traffic at no cost).

Symmetry trick: the pairwise matrix is symmetric in (i,j) for both the
real and imaginary components (O[i,j] = O[j,i]), so each channel only
computes 32-row i-blocks against j >= 32*bi — 56.25% of the matrix —
and the host mirrors the lower block-triangle for free.

Sharding: 8 cores = 2 batches x 4 tau-quarters.  Each core owns 32
channels ch = ell*8 + s (t = tq*8 + s).  Four channels (a quad) share
each matmul's 128 PSUM partitions via column tiling (tile_position) —
the four streams run concurrently on disjoint 32-column PE tiles, so a
quad i-block costs one W-column pass.  Each ell lives in its own
32-partition row group (rows 32*ell + [0, 2m)).  The eight i-blocks of a
quad are packed into five single-PSUM-bank tiles — (0), (1,7), (2,6),
(3,5), (4) — so evacuation is five wide copies (f32 -> fp16) with a
fixed ScalarE/VectorE split sized to each engine's measured per-column
rate.  Two quads share each [128, 4608] staging tile so output stores
are 9216B-per-partition-line DMAs (small lines throttle the SDMA
engines), issued on the sync ring with the last pair on the scalar ring.
Host reassembles the full [2,256,256,128,2] output.
"""

import numpy as np
import ml_dtypes

import concourse.bass as bass
import concourse.bacc as bacc
import concourse.mybir as mybir
from concourse.bass_utils import run_bass_kernel_spmd
from concourse.tile import TileContext

B, N, TAU, NELL = 2, 256, 32, 4
NCORES = 8
NCH = 32          # channels per core (4 ell * 8 slots)
F32 = mybir.dt.float32
F16 = mybir.dt.float16
BF16 = mybir.dt.bfloat16
BFNP = ml_dtypes.bfloat16
KS = [2 * (2 * ell + 1) for ell in range(NELL)]       # 2, 6, 10, 14
# contraction dims padded with zero rows to a multiple of 4: the PE
# streams bf16 moving data at half rate when K is not 4-aligned
KP = [4, 8, 12, 16]
BIW = [512 - 64 * bi for bi in range(8)]              # cols per 32-row i-block
# PSUM pack: one 2-bank tile, two single-bank tiles, one single-bank
# tile per quad.  Each i-block sits inside one 2KB bank; each PACK
# group fills its tile exactly (512+448+64 = 1024, 384+128 = 320+192 =
# 512, 256) so evacuation is one wide copy per tile.
PACK = [(0, 1, 7), (2, 6), (3, 5), (4,)]              # widths 1024, 512, 512, 256
PACKW = [sum(BIW[b] for b in g) for g in PACK]
# offset of each i-block inside the quad's 2304-col staging span
BIO2 = {}
_off = 0
for _g in PACK:
    for _b in _g:
        BIO2[_b] = _off
        _off += BIW[_b]
OTW = 2304                                            # sum of all widths
QUAD_ORDER = [(0, 0), (0, 1), (3, 0), (3, 1), (2, 0), (2, 1), (1, 0), (1, 1)]
IN_COLS = 8 * 256 + 8 * 512                           # 6144: lhs slots then rhs slots

_NC_CACHE = {}


def _build_bass():
    nc = bacc.Bacc()
    # One input tensor per ell: [2m, 6144] bf16; cols [0:2048) hold the
    # eight 256-wide lhsT slot blocks, [2048:6144) the eight 512-wide rhs
    # slot blocks.  Each lands in SBUF row group 32*ell.
    inps = [
        nc.declare_dram_parameter(f"inp{e}", [KP[e], IN_COLS], BF16, isOutput=False)
        for e in range(NELL)
    ]
    # Output: eight 2304-col quad spans side by side; stores cover
    # single quads or pairs depending on position in the schedule
    out = nc.declare_dram_parameter("out", [128, 8 * OTW], F16, isOutput=True)

    with TileContext(nc) as tc:
        with (
            tc.tile_pool(name="lin", bufs=1) as lin_pool,
            tc.tile_pool(name="in0", bufs=1) as in0_pool,
            tc.tile_pool(name="in1", bufs=1) as in1_pool,
            tc.tile_pool(name="in2", bufs=1) as in2_pool,
            tc.tile_pool(name="in3", bufs=1) as in3_pool,
            tc.tile_pool(name="psA", bufs=2, space="PSUM") as psA_pool,
            tc.tile_pool(name="psB", bufs=3, space="PSUM") as psB_pool,
            tc.tile_pool(name="psC", bufs=1, space="PSUM") as psC_pool,
            tc.tile_pool(name="ot1", bufs=2) as ot1_pool,
            tc.tile_pool(name="ot2", bufs=3) as ot2_pool,
        ):
            # One SBUF tile per ell, each in its own pool: dependency
            # tracking is pool-slot-granular, so sharing a pool would make
            # the FIRST matmul wait for the LAST input DMA.  Separate
            # pools let quad 0 start as soon as ell0's load lands, with
            # later ells streaming in under compute.
            in_pools = [in0_pool, in1_pool, in2_pool, in3_pool]
            in_sbs = [
                in_pools[e].tile([128, IN_COLS], BF16, name=f"in_sb{e}")
                for e in range(NELL)
            ]
            # PE pre-warm: dependency-free dummy matmuls keep the PE busy
            # from kernel start until the first input lands — a PE idle
            # gap here drops the HAM clock gate and the whole kernel then
            # streams at half rate.
            warm_in = lin_pool.tile([128, 512], BF16, name="warm_in")
            warm_ps = psC_pool.tile([128, 512], F32, tag="psC", name="warm_ps")
            nc.vector.memset(warm_in[:], 0.0)
            # The first real matmul cannot start before ~11.8us (input
            # completion semaphore latency is fixed regardless of input
            # size); eight warmups bridge the whole window so the PE never
            # idles and the clock stays at full rate.
            for _ in range(8):
                nc.tensor.matmul(
                    warm_ps[:, 0:512], warm_in[:, 0:128], warm_in[:, 0:512],
                    start=True, stop=True,
                )
            # Split the four input loads across both HWDGE rings so their
            # ~700ns descriptor generations run in parallel: every matmul
            # waits on the full input set, so last-input-landed is what
            # gates compute start.
            nc.sync.dma_start(out=in_sbs[0][0 : KP[0], :], in_=inps[0][:])
            nc.scalar.dma_start(out=in_sbs[3][0 : KP[3], :], in_=inps[3][:])
            nc.sync.dma_start(out=in_sbs[2][0 : KP[2], :], in_=inps[2][:])
            nc.scalar.dma_start(out=in_sbs[1][0 : KP[1], :], in_=inps[1][:])
            # store schedule: (0), (1,2), (3,4), (5,6), (7) — the early
            # single gets the store stream flowing ~2us sooner, the pairs
            # keep 9216B partition lines for full SDMA packet efficiency
            ot = None
            for qidx, (e, v) in enumerate(QUAD_ORDER):
                K = KP[e]
                in_sb = in_sbs[e]
                if qidx in (0, 7):
                    ot = ot1_pool.tile([128, OTW], F16)
                    qoff = 0
                elif qidx in (1, 3, 5):
                    ot = ot2_pool.tile([128, 2 * OTW], F16)
                    qoff = 0
                else:
                    qoff = OTW
                for gi, grp in enumerate(PACK):
                    if gi == 0:
                        ps = psA_pool.tile([128, 1024], F32, name="psa")
                    elif gi == 3:
                        ps = psC_pool.tile([128, 512], F32, tag="psC", name="psc")
                    else:
                        ps = psB_pool.tile([128, 512], F32, name="psb")
                    poff = 0
                    for bi in grp:
                        W = BIW[bi]
                        for c4 in range(4):  # channel within quad
                            sl = v * 4 + c4
                            lo = sl * 256
                            ro = 2048 + sl * 512
                            nc.tensor.matmul(
                                ps[c4 * 32 : (c4 + 1) * 32, poff : poff + W],
                                in_sb[0:K, lo + bi * 32 : lo + bi * 32 + 32],
                                in_sb[0:K, ro + 64 * bi : ro + 512],
                                start=True,
                                stop=True,
                                tile_position=(0, c4 * 32),
                            )
                        poff += W
                    base = qoff + BIO2[grp[0]]
                    # one wide copy per tile; scalar (the faster PSUM
                    # reader) takes the 1024-col tile + the small tile,
                    # vector the two 512-col tiles — ~1.48us vs ~1.25us
                    # chains per quad
                    if gi in (1, 2):
                        nc.vector.tensor_copy(
                            out=ot[:, base : base + PACKW[gi]], in_=ps[:, 0 : PACKW[gi]]
                        )
                    else:
                        nc.scalar.copy(
                            ot[:, base : base + PACKW[gi]], ps[:, 0 : PACKW[gi]]
                        )
                if qidx in (0, 2, 4, 6, 7):
                    # sync ring carries everything except the final single,
                    # which rides the scalar ring (its copies are done by
                    # then) so the last two transfers drain concurrently
                    span = OTW if qidx in (0, 7) else 2 * OTW
                    dst0 = qidx * OTW if qidx in (0, 7) else (qidx - 1) * OTW
                    eng = nc.scalar if qidx == 7 else nc.sync
                    eng.dma_start(out=out[:, dst0 : dst0 + span], in_=ot[:])
    nc.compile()
    return nc


def _host_prep(reps, cid):
    """Build per-core bf16 lhsT/rhs input tensors (one per ell)."""
    b, tq = cid // 4, cid % 4
    im = {}
    for ell in range(NELL):
        rep = reps[ell]
        m = 2 * ell + 1
        s_vec = ((-1.0) ** (ell + np.arange(m))).astype(np.float32)
        arr = np.zeros((KP[ell], IN_COLS), np.float32)
        for sidx in range(8):
            t = tq * 8 + sidx
            Z = rep[b, :, t]                      # [256, m, 2]
            Zr, Zi = Z[..., 0], Z[..., 1]         # [256, m]
            arr[0:m, sidx * 256 : sidx * 256 + 256] = Zr.T
            arr[m : 2 * m, sidx * 256 : sidx * 256 + 256] = Zi.T
            FZr = s_vec[:, None] * Zr[:, ::-1].T             # [m, 256]
            FZi = s_vec[:, None] * Zi[:, ::-1].T
            R = np.empty((2 * m, 256, 2), np.float32)
            R[0:m, :, 0] = FZr
            R[m:, :, 0] = -FZi
            R[0:m, :, 1] = FZi
            R[m:, :, 1] = FZr
            ro = 2048 + sidx * 512
            arr[0 : 2 * m, ro : ro + 512] = R.reshape(2 * m, 512)
        im[f"inp{ell}"] = arr.astype(BFNP)
    return im


def _run(in_maps, **kw):
    if "nc" not in _NC_CACHE:
        _NC_CACHE["nc"] = _build_bass()
    return run_bass_kernel_spmd(_NC_CACHE["nc"], in_maps, list(range(NCORES)), **kw)


def kernel(rep0, rep1, rep2, rep3, _bass_kw=None):
    reps = [np.ascontiguousarray(np.asarray(r, dtype=np.float32)) for r in (rep0, rep1, rep2, rep3)]
    in_maps = [_host_prep(reps, cid) for cid in range(NCORES)]
    res = _run(in_maps, **(_bass_kw or {}))
    out = np.empty((B, N, N, NELL * TAU, 2), np.float32)
    for cid in range(NCORES):
        b, tq = cid // 4, cid % 4
        arr = res.results[cid]["out"]          # [128, 8*2304] fp16
        o = np.empty((NELL, 8, 256, 256, 2), np.float32)   # [ell, slot, i, j, c]
        for qidx, (e, v) in enumerate(QUAD_ORDER):
            a = arr[:, qidx * OTW : (qidx + 1) * OTW].astype(np.float32)
            for bi in range(8):
                nj = 256 - 32 * bi
                blk = a[:, BIO2[bi] : BIO2[bi] + BIW[bi]].reshape(4, 32, nj, 2)
                for c4 in range(4):
                    o[e, 4 * v + c4, 32 * bi : 32 * bi + 32, 32 * bi :, :] = blk[c4]
        for bi in range(1, 8):                  # mirror lower block triangle
            r = slice(32 * bi, 32 * bi + 32)
            o[:, :, r, : 32 * bi, :] = o[:, :, : 32 * bi, r, :].transpose(0, 1, 3, 2, 4)
        for e in range(NELL):
            lo = e * TAU + tq * 8
            out[b, :, :, lo : lo + 8, :] = o[e].transpose(1, 2, 0, 3)
    kernel.last_result = res
    return out


# revision 36
# speedup vs baseline: 1.0949x; 1.0627x over previous
"""Trainium2 Bass kernel for nn_DotMatrix.

Math: for each (b, ell, t) the reference computes a complex pairwise dot
matrix O[i,j] = sum_m z[i,m] * w[j,m] where z = rep[b,:,t,:,:] as complex
and w the sign-flipped conjugation partner.  As a real matmul:

  lhsT[k, i]   k = (c,m) stacked: [Zr.T; Zi.T]                 [2m, 256]
  rhs[k, 2j+c'] c'=0: [FZr; -FZi], c'=1: [FZi; FZr]            [2m, 512]
  out = lhsT.T @ rhs  -> [256 i, 512 (j,c)]

with FZr[m',j] = s[m'] * Zr[j, M-1-m'], s[m'] = (-1)^(ell+m').

Precision: bf16 operands with fp32 PSUM accumulation and fp16 stores give
~3e-3 relative error on the final output (gate is 2e-2), so no multi-term
decomposition is needed — the contraction dim stays at K = 2m <= 14, the
input tensors total just 393KB per core, and the PE streams one column
per cycle (fp16 operands would halve that rate; fp16 is only used on the
store side where it halves HBM traffic at no cost).

Symmetry trick: the pairwise matrix is symmetric in (i,j) for both the
real and imaginary components (O[i,j] = O[j,i]), so each channel only
computes 32-row i-blocks against j >= 32*bi — 56.25% of the matrix —
and the host mirrors the lower block-triangle for free.

Sharding: 8 cores = 2 batches x 4 tau-quarters.  Each core owns 32
channels ch = ell*8 + s (t = tq*8 + s).  Four channels (a quad) share
each matmul's 128 PSUM partitions via column tiling (tile_position) —
the four streams run concurrently on disjoint 32-column PE tiles, so a
quad i-block costs one W-column pass.  Each ell lives in its own
32-partition row group (rows 32*ell + [0, 2m)).  The eight i-blocks of a
quad are packed into five single-PSUM-bank tiles — (0), (1,7), (2,6),
(3,5), (4) — so evacuation is five wide copies (f32 -> fp16) with a
fixed ScalarE/VectorE split sized to each engine's measured per-column
rate.  Two quads share each [128, 4608] staging tile so output stores
are 9216B-per-partition-line DMAs (small lines throttle the SDMA
engines), issued on the sync ring with the last pair on the scalar ring.
Host reassembles the full [2,256,256,128,2] output.
"""

import numpy as np
import ml_dtypes

import concourse.bass as bass
import concourse.bacc as bacc
import concourse.mybir as mybir
from concourse.bass_utils import run_bass_kernel_spmd
from concourse.tile import TileContext

B, N, TAU, NELL = 2, 256, 32, 4
NCORES = 8
NCH = 32          # channels per core (4 ell * 8 slots)
F32 = mybir.dt.float32
F16 = mybir.dt.float16
BF16 = mybir.dt.bfloat16
BFNP = ml_dtypes.bfloat16
KS = [2 * (2 * ell + 1) for ell in range(NELL)]       # 2, 6, 10, 14
# contraction dims padded with zero rows to a multiple of 4: the PE
# streams bf16 moving data at half rate when K is not 4-aligned
KP = [4, 8, 12, 16]
BIW = [512 - 64 * bi for bi in range(8)]              # cols per 32-row i-block
# PSUM pack: one 2-bank tile, two single-bank tiles, one single-bank
# tile per quad.  Each i-block sits inside one 2KB bank; each PACK
# group fills its tile exactly (512+448+64 = 1024, 384+128 = 320+192 =
# 512, 256) so evacuation is one wide copy per tile.
PACK = [(0, 1, 7), (2, 6), (3, 5), (4,)]              # widths 1024, 512, 512, 256
PACKW = [sum(BIW[b] for b in g) for g in PACK]
# offset of each i-block inside the quad's 2304-col staging span
BIO2 = {}
_off = 0
for _g in PACK:
    for _b in _g:
        BIO2[_b] = _off
        _off += BIW[_b]
OTW = 2304                                            # sum of all widths
QUAD_ORDER = [(0, 0), (0, 1), (3, 0), (3, 1), (2, 0), (2, 1), (1, 0), (1, 1)]
IN_COLS = 8 * 256 + 8 * 512                           # 6144: lhs slots then rhs slots

_NC_CACHE = {}


def _build_bass():
    nc = bacc.Bacc()
    # One input tensor per ell: [2m, 6144] bf16; cols [0:2048) hold the
    # eight 256-wide lhsT slot blocks, [2048:6144) the eight 512-wide rhs
    # slot blocks.  Each lands in SBUF row group 32*ell.
    inps = [
        nc.declare_dram_parameter(f"inp{e}", [KP[e], IN_COLS], BF16, isOutput=False)
        for e in range(NELL)
    ]
    # Output: eight 2304-col quad spans side by side; stores cover
    # single quads or pairs depending on position in the schedule
    out = nc.declare_dram_parameter("out", [128, 8 * OTW], F16, isOutput=True)

    with TileContext(nc) as tc:
        with (
            tc.tile_pool(name="lin", bufs=1) as lin_pool,
            tc.tile_pool(name="in0", bufs=1) as in0_pool,
            tc.tile_pool(name="in1", bufs=1) as in1_pool,
            tc.tile_pool(name="in2", bufs=1) as in2_pool,
            tc.tile_pool(name="in3", bufs=1) as in3_pool,
            tc.tile_pool(name="psA", bufs=2, space="PSUM") as psA_pool,
            tc.tile_pool(name="psB", bufs=3, space="PSUM") as psB_pool,
            tc.tile_pool(name="psC", bufs=1, space="PSUM") as psC_pool,
            tc.tile_pool(name="ot1", bufs=4) as ot1_pool,
        ):
            # One SBUF tile per ell, each in its own pool: dependency
            # tracking is pool-slot-granular, so sharing a pool would make
            # the FIRST matmul wait for the LAST input DMA.  Separate
            # pools let quad 0 start as soon as ell0's load lands, with
            # later ells streaming in under compute.
            in_pools = [in0_pool, in1_pool, in2_pool, in3_pool]
            in_sbs = [
                in_pools[e].tile([128, IN_COLS], BF16, name=f"in_sb{e}")
                for e in range(NELL)
            ]
            # PE pre-warm: dependency-free dummy matmuls keep the PE busy
            # from kernel start until the first input lands — a PE idle
            # gap here drops the HAM clock gate and the whole kernel then
            # streams at half rate.
            warm_in = lin_pool.tile([128, 512], BF16, name="warm_in")
            warm_ps = psC_pool.tile([128, 512], F32, tag="psC", name="warm_ps")
            nc.vector.memset(warm_in[:], 0.0)
            # The first real matmul cannot start before ~11.8us (input
            # completion semaphore latency is fixed regardless of input
            # size); eight warmups bridge the whole window so the PE never
            # idles and the clock stays at full rate.
            for _ in range(8):
                nc.tensor.matmul(
                    warm_ps[:, 0:512], warm_in[:, 0:128], warm_in[:, 0:512],
                    start=True, stop=True,
                )
            # Split the four input loads across both HWDGE rings so their
            # ~700ns descriptor generations run in parallel: every matmul
            # waits on the full input set, so last-input-landed is what
            # gates compute start.
            nc.sync.dma_start(out=in_sbs[0][0 : KP[0], :], in_=inps[0][:])
            nc.scalar.dma_start(out=in_sbs[3][0 : KP[3], :], in_=inps[3][:])
            nc.sync.dma_start(out=in_sbs[2][0 : KP[2], :], in_=inps[2][:])
            nc.scalar.dma_start(out=in_sbs[1][0 : KP[1], :], in_=inps[1][:])
            # per-quad stores alternating rings: the stream starts as soon
            # as the first quad is evacuated, and the two rings drain
            # concurrently
            ot = None
            for qidx, (e, v) in enumerate(QUAD_ORDER):
                K = KP[e]
                in_sb = in_sbs[e]
                ot = ot1_pool.tile([128, OTW], F16)
                qoff = 0
                for gi, grp in enumerate(PACK):
                    if gi == 0:
                        ps = psA_pool.tile([128, 1024], F32, name="psa")
                    elif gi == 3:
                        ps = psC_pool.tile([128, 512], F32, tag="psC", name="psc")
                    else:
                        ps = psB_pool.tile([128, 512], F32, name="psb")
                    poff = 0
                    for bi in grp:
                        W = BIW[bi]
                        for c4 in range(4):  # channel within quad
                            sl = v * 4 + c4
                            lo = sl * 256
                            ro = 2048 + sl * 512
                            nc.tensor.matmul(
                                ps[c4 * 32 : (c4 + 1) * 32, poff : poff + W],
                                in_sb[0:K, lo + bi * 32 : lo + bi * 32 + 32],
                                in_sb[0:K, ro + 64 * bi : ro + 512],
                                start=True,
                                stop=True,
                                tile_position=(0, c4 * 32),
                            )
                        poff += W
                    base = qoff + BIO2[grp[0]]
                    # one wide copy per tile; scalar (the faster PSUM
                    # reader) takes the 1024-col tile + the small tile,
                    # vector the two 512-col tiles — ~1.48us vs ~1.25us
                    # chains per quad
                    if gi in (1, 2):
                        nc.vector.tensor_copy(
                            out=ot[:, base : base + PACKW[gi]], in_=ps[:, 0 : PACKW[gi]]
                        )
                    else:
                        nc.scalar.copy(
                            ot[:, base : base + PACKW[gi]], ps[:, 0 : PACKW[gi]]
                        )
                eng = nc.sync if qidx % 2 == 0 else nc.scalar
                eng.dma_start(
                    out=out[:, qidx * OTW : (qidx + 1) * OTW], in_=ot[:]
                )
    nc.compile()
    return nc


def _host_prep(reps, cid):
    """Build per-core bf16 lhsT/rhs input tensors (one per ell)."""
    b, tq = cid // 4, cid % 4
    im = {}
    for ell in range(NELL):
        rep = reps[ell]
        m = 2 * ell + 1
        s_vec = ((-1.0) ** (ell + np.arange(m))).astype(np.float32)
        arr = np.zeros((KP[ell], IN_COLS), np.float32)
        for sidx in range(8):
            t = tq * 8 + sidx
            Z = rep[b, :, t]                      # [256, m, 2]
            Zr, Zi = Z[..., 0], Z[..., 1]         # [256, m]
            arr[0:m, sidx * 256 : sidx * 256 + 256] = Zr.T
            arr[m : 2 * m, sidx * 256 : sidx * 256 + 256] = Zi.T
            FZr = s_vec[:, None] * Zr[:, ::-1].T             # [m, 256]
            FZi = s_vec[:, None] * Zi[:, ::-1].T
            R = np.empty((2 * m, 256, 2), np.float32)
            R[0:m, :, 0] = FZr
            R[m:, :, 0] = -FZi
            R[0:m, :, 1] = FZi
            R[m:, :, 1] = FZr
            ro = 2048 + sidx * 512
            arr[0 : 2 * m, ro : ro + 512] = R.reshape(2 * m, 512)
        im[f"inp{ell}"] = arr.astype(BFNP)
    return im


def _run(in_maps, **kw):
    if "nc" not in _NC_CACHE:
        _NC_CACHE["nc"] = _build_bass()
    return run_bass_kernel_spmd(_NC_CACHE["nc"], in_maps, list(range(NCORES)), **kw)


def kernel(rep0, rep1, rep2, rep3, _bass_kw=None):
    reps = [np.ascontiguousarray(np.asarray(r, dtype=np.float32)) for r in (rep0, rep1, rep2, rep3)]
    in_maps = [_host_prep(reps, cid) for cid in range(NCORES)]
    res = _run(in_maps, **(_bass_kw or {}))
    out = np.empty((B, N, N, NELL * TAU, 2), np.float32)
    for cid in range(NCORES):
        b, tq = cid // 4, cid % 4
        arr = res.results[cid]["out"]          # [128, 8*2304] fp16
        o = np.empty((NELL, 8, 256, 256, 2), np.float32)   # [ell, slot, i, j, c]
        for qidx, (e, v) in enumerate(QUAD_ORDER):
            a = arr[:, qidx * OTW : (qidx + 1) * OTW].astype(np.float32)
            for bi in range(8):
                nj = 256 - 32 * bi
                blk = a[:, BIO2[bi] : BIO2[bi] + BIW[bi]].reshape(4, 32, nj, 2)
                for c4 in range(4):
                    o[e, 4 * v + c4, 32 * bi : 32 * bi + 32, 32 * bi :, :] = blk[c4]
        for bi in range(1, 8):                  # mirror lower block triangle
            r = slice(32 * bi, 32 * bi + 32)
            o[:, :, r, : 32 * bi, :] = o[:, :, : 32 * bi, r, :].transpose(0, 1, 3, 2, 4)
        for e in range(NELL):
            lo = e * TAU + tq * 8
            out[b, :, :, lo : lo + 8, :] = o[e].transpose(1, 2, 0, 3)
    kernel.last_result = res
    return out


# revision 44
# speedup vs baseline: 1.1009x; 1.0054x over previous
"""Trainium2 Bass kernel for nn_DotMatrix.

Math: for each (b, ell, t) the reference computes a complex pairwise dot
matrix O[i,j] = sum_m z[i,m] * w[j,m] where z = rep[b,:,t,:,:] as complex
and w the sign-flipped conjugation partner.  As a real matmul:

  lhsT[k, i]   k = (c,m) stacked: [Zr.T; Zi.T]                 [2m, 256]
  rhs[k, 2j+c'] c'=0: [FZr; -FZi], c'=1: [FZi; FZr]            [2m, 512]
  out = lhsT.T @ rhs  -> [256 i, 512 (j,c)]

with FZr[m',j] = s[m'] * Zr[j, M-1-m'], s[m'] = (-1)^(ell+m').

Precision: fp16 operands with fp32 PSUM accumulation and fp16 stores
give ~4e-4 relative error on the final output (gate is 2e-2), so no
multi-term decomposition is needed — the contraction dim stays at
K = 2m <= 14 (zero-padded to a multiple of 4) and the input tensors
total just ~0.5MB per core.  The PE streams each tile's moving data at
~0.82ns/column with all four column-tiles of a block concurrent, so a
block costs one max(W)-column pass regardless of dtype — the measured
floor for this output volume.

Symmetry trick: the pairwise matrix is symmetric in (i,j) for both the
real and imaginary components (O[i,j] = O[j,i]), so each channel only
computes 32-row i-blocks against j >= 32*bi — 56.25% of the matrix —
and the host mirrors the lower block-triangle for free.

Sharding: 8 cores = 2 batches x 4 tau-quarters.  Each core owns 32
channels ch = ell*8 + s (t = tq*8 + s).  Four channels (a quad) share
each matmul's 128 PSUM partitions via column tiling (tile_position) —
the four streams run concurrently on disjoint 32-column PE tiles, so a
quad i-block costs one W-column pass.  Each ell's inputs live in their
own SBUF tile in a dedicated pool (dependency tracking is pool-slot
granular — a shared pool would gate the first matmul on the last input
DMA), loaded via both HWDGE rings in consumption order; eight dummy
matmuls bridge the fixed ~3.5us input-DMA semaphore latency so the PE
never idles.  The eight i-blocks of a quad are packed into four PSUM
tiles — (0,1,7) in two banks, (2,6), (3,5), (4) in one each, every
i-block inside a single 2KB bank — so evacuation is one wide copy per
tile (f32 -> fp16), split ScalarE {1024+256 cols} / VectorE {512+512}
to balance the two engines' chains.  Each quad leaves as one [128,2304]
fp16 store, alternating the sync/scalar rings so the output stream
starts as early as possible and drains on both rings.  Host reassembles
the full [2,256,256,128,2] output.
"""

import numpy as np

import concourse.bacc as bacc
import concourse.mybir as mybir
from concourse.bass_utils import run_bass_kernel_spmd
from concourse.tile import TileContext

B, N, TAU, NELL = 2, 256, 32, 4
NCORES = 8
NCH = 32          # channels per core (4 ell * 8 slots)
F32 = mybir.dt.float32
F16 = mybir.dt.float16
KS = [2 * (2 * ell + 1) for ell in range(NELL)]       # 2, 6, 10, 14
# contraction dims padded with zero rows to a multiple of 4 so moving
# data stays 8B-aligned per column
KP = [4, 8, 12, 16]
BIW = [512 - 64 * bi for bi in range(8)]              # cols per 32-row i-block
# PSUM pack: one 2-bank tile, two single-bank tiles, one single-bank
# tile per quad.  Each i-block sits inside one 2KB bank; each PACK
# group fills its tile exactly (512+448+64 = 1024, 384+128 = 320+192 =
# 512, 256) so evacuation is one wide copy per tile.
PACK = [(0, 1, 7), (2, 6), (3, 5), (4,)]              # widths 1024, 512, 512, 256
PACKW = [sum(BIW[b] for b in g) for g in PACK]
# offset of each i-block inside the quad's 2304-col staging span
BIO2 = {}
_off = 0
for _g in PACK:
    for _b in _g:
        BIO2[_b] = _off
        _off += BIW[_b]
OTW = 2304                                            # sum of all widths
QUAD_ORDER = [(0, 0), (0, 1), (3, 0), (3, 1), (2, 0), (2, 1), (1, 0), (1, 1)]
IN_COLS = 8 * 256 + 8 * 512                           # 6144: lhs slots then rhs slots

_NC_CACHE = {}


def _build_bass():
    nc = bacc.Bacc()
    # One input tensor per ell: [KP, 6144] fp16; cols [0:2048) hold the
    # eight 256-wide lhsT slot blocks, [2048:6144) the eight 512-wide rhs
    # slot blocks.
    inps = [
        nc.declare_dram_parameter(f"inp{e}", [KP[e], IN_COLS], F16, isOutput=False)
        for e in range(NELL)
    ]
    # Output: eight 2304-col quad spans side by side
    out = nc.declare_dram_parameter("out", [128, 8 * OTW], F16, isOutput=True)

    with TileContext(nc) as tc:
        with (
            tc.tile_pool(name="lin", bufs=1) as lin_pool,
            tc.tile_pool(name="in0", bufs=1) as in0_pool,
            tc.tile_pool(name="in1", bufs=1) as in1_pool,
            tc.tile_pool(name="in2", bufs=1) as in2_pool,
            tc.tile_pool(name="in3", bufs=1) as in3_pool,
            tc.tile_pool(name="psA", bufs=2, space="PSUM") as psA_pool,
            tc.tile_pool(name="psB", bufs=3, space="PSUM") as psB_pool,
            tc.tile_pool(name="psC", bufs=1, space="PSUM") as psC_pool,
            tc.tile_pool(name="ot1", bufs=4) as ot1_pool,
        ):
            # One SBUF tile per ell, each in its own pool: dependency
            # tracking is pool-slot-granular, so sharing a pool would make
            # the FIRST matmul wait for the LAST input DMA.  Separate
            # pools let quad 0 start as soon as ell0's load lands, with
            # later ells streaming in under compute.
            in_pools = [in0_pool, in1_pool, in2_pool, in3_pool]
            in_sbs = [
                in_pools[e].tile([128, IN_COLS], F16, name=f"in_sb{e}")
                for e in range(NELL)
            ]
            # PE pre-warm: dependency-free dummy matmuls keep the PE busy
            # from kernel start until the first input lands — a PE idle
            # gap here drops the HAM clock gate and the whole kernel then
            # streams at half rate.
            warm_in = lin_pool.tile([128, 512], F16, name="warm_in")
            warm_ps = psC_pool.tile([128, 512], F32, tag="psC", name="warm_ps")
            nc.vector.memset(warm_in[:], 0.0)
            # The first real matmul cannot start before ~11.8us (input
            # completion semaphore latency is fixed regardless of input
            # size); eight warmups bridge the whole window so the PE never
            # idles and the clock stays at full rate.
            for _ in range(8):
                nc.tensor.matmul(
                    warm_ps[:, 0:512], warm_in[:, 0:128], warm_in[:, 0:512],
                    start=True, stop=True,
                )
            # Split the four input loads across both HWDGE rings so their
            # ~700ns descriptor generations run in parallel: every matmul
            # waits on the full input set, so last-input-landed is what
            # gates compute start.
            nc.sync.dma_start(out=in_sbs[0][0 : KP[0], :], in_=inps[0][:])
            nc.scalar.dma_start(out=in_sbs[3][0 : KP[3], :], in_=inps[3][:])
            nc.sync.dma_start(out=in_sbs[2][0 : KP[2], :], in_=inps[2][:])
            nc.scalar.dma_start(out=in_sbs[1][0 : KP[1], :], in_=inps[1][:])
            # per-quad stores alternating rings: the stream starts as soon
            # as the first quad is evacuated, and the two rings drain
            # concurrently
            ot = None
            for qidx, (e, v) in enumerate(QUAD_ORDER):
                K = KP[e]
                in_sb = in_sbs[e]
                ot = ot1_pool.tile([128, OTW], F16)
                qoff = 0
                for gi, grp in enumerate(PACK):
                    if gi == 0:
                        ps = psA_pool.tile([128, 1024], F32, name="psa")
                    elif gi == 3:
                        ps = psC_pool.tile([128, 512], F32, tag="psC", name="psc")
                    else:
                        ps = psB_pool.tile([128, 512], F32, name="psb")
                    poff = 0
                    for bi in grp:
                        W = BIW[bi]
                        for c4 in range(4):  # channel within quad
                            sl = v * 4 + c4
                            lo = sl * 256
                            ro = 2048 + sl * 512
                            nc.tensor.matmul(
                                ps[c4 * 32 : (c4 + 1) * 32, poff : poff + W],
                                in_sb[0:K, lo + bi * 32 : lo + bi * 32 + 32],
                                in_sb[0:K, ro + 64 * bi : ro + 512],
                                start=True,
                                stop=True,
                                tile_position=(0, c4 * 32),
                            )
                        poff += W
                    base = qoff + BIO2[grp[0]]
                    # one wide copy per tile; scalar (the faster PSUM
                    # reader) takes the 1024-col tile + the small tile,
                    # vector the two 512-col tiles — ~1.48us vs ~1.25us
                    # chains per quad
                    if gi in (1, 2):
                        nc.vector.tensor_copy(
                            out=ot[:, base : base + PACKW[gi]], in_=ps[:, 0 : PACKW[gi]]
                        )
                    else:
                        nc.scalar.copy(
                            ot[:, base : base + PACKW[gi]], ps[:, 0 : PACKW[gi]]
                        )
                eng = nc.sync if qidx % 2 == 0 else nc.scalar
                eng.dma_start(
                    out=out[:, qidx * OTW : (qidx + 1) * OTW], in_=ot[:]
                )
    nc.compile()
    return nc


def _host_prep(reps, cid):
    """Build per-core fp16 lhsT/rhs input tensors (one per ell)."""
    b, tq = cid // 4, cid % 4
    im = {}
    for ell in range(NELL):
        rep = reps[ell]
        m = 2 * ell + 1
        s_vec = ((-1.0) ** (ell + np.arange(m))).astype(np.float32)
        arr = np.zeros((KP[ell], IN_COLS), np.float32)
        for sidx in range(8):
            t = tq * 8 + sidx
            Z = rep[b, :, t]                      # [256, m, 2]
            Zr, Zi = Z[..., 0], Z[..., 1]         # [256, m]
            arr[0:m, sidx * 256 : sidx * 256 + 256] = Zr.T
            arr[m : 2 * m, sidx * 256 : sidx * 256 + 256] = Zi.T
            FZr = s_vec[:, None] * Zr[:, ::-1].T             # [m, 256]
            FZi = s_vec[:, None] * Zi[:, ::-1].T
            R = np.empty((2 * m, 256, 2), np.float32)
            R[0:m, :, 0] = FZr
            R[m:, :, 0] = -FZi
            R[0:m, :, 1] = FZi
            R[m:, :, 1] = FZr
            ro = 2048 + sidx * 512
            arr[0 : 2 * m, ro : ro + 512] = R.reshape(2 * m, 512)
        im[f"inp{ell}"] = arr.astype(np.float16)
    return im


def _run(in_maps, **kw):
    if "nc" not in _NC_CACHE:
        _NC_CACHE["nc"] = _build_bass()
    return run_bass_kernel_spmd(_NC_CACHE["nc"], in_maps, list(range(NCORES)), **kw)


def kernel(rep0, rep1, rep2, rep3, _bass_kw=None):
    reps = [np.ascontiguousarray(np.asarray(r, dtype=np.float32)) for r in (rep0, rep1, rep2, rep3)]
    in_maps = [_host_prep(reps, cid) for cid in range(NCORES)]
    res = _run(in_maps, **(_bass_kw or {}))
    out = np.empty((B, N, N, NELL * TAU, 2), np.float32)
    for cid in range(NCORES):
        b, tq = cid // 4, cid % 4
        arr = res.results[cid]["out"]          # [128, 8*2304] fp16
        o = np.empty((NELL, 8, 256, 256, 2), np.float32)   # [ell, slot, i, j, c]
        for qidx, (e, v) in enumerate(QUAD_ORDER):
            a = arr[:, qidx * OTW : (qidx + 1) * OTW].astype(np.float32)
            for bi in range(8):
                nj = 256 - 32 * bi
                blk = a[:, BIO2[bi] : BIO2[bi] + BIW[bi]].reshape(4, 32, nj, 2)
                for c4 in range(4):
                    o[e, 4 * v + c4, 32 * bi : 32 * bi + 32, 32 * bi :, :] = blk[c4]
        for bi in range(1, 8):                  # mirror lower block triangle
            r = slice(32 * bi, 32 * bi + 32)
            o[:, :, r, : 32 * bi, :] = o[:, :, : 32 * bi, r, :].transpose(0, 1, 3, 2, 4)
        for e in range(NELL):
            lo = e * TAU + tq * 8
            out[b, :, :, lo : lo + 8, :] = o[e].transpose(1, 2, 0, 3)
    kernel.last_result = res
    return out
